# revision 23
# baseline (speedup 1.0000x reference)
"""GCN 2-layer encoder on 8 Trainium2 NeuronCores (Bass/Tile) — fused single
launch.

kernel(**inputs) takes the FULL inputs and returns the FULL [80000, 32] f32
output.  Strategy (node partition across 8 cores, per sharding hint):

  gcn_conv(x, W, b) = b + dinv * (A_hat @ (dinv * (x @ W)))  with self-loops,
  where dinv = 1/sqrt(indeg+1) and A_hat is the (unnormalized) adjacency.

One SPMD program per core does all of:
  A:   z1 = dinv * (x @ W1) on the core's natural node shard -> zloc (DRAM)
  AG1: AllGather zloc -> zfull  (on-device HBM collective)
  L1:  per dst-node-tile (128 nodes, load-balanced rank order) gather zfull
       rows by edge source (gpsimd dma_gather, 256B rows) and reduce with a
       one-hot scatter-matmul on the PE into PSUM; epilogue
       z2 = relu(dinv^2*agg + dinv*b1);  y = z2 @ W2  -> yloc (DRAM)
  AG2: AllGather yloc -> yfull
  L2:  same aggregation over yfull; epilogue out = dinv*agg + b2.

Cross-partition edge exchange happens entirely on-device via the two
AllGathers, so the whole 2-layer GCN is one kernel launch.  Edges are grouped
by (dst tile, src range) with counts padded to a global per-rank max so all 8
cores run one identical SPMD program; int16 gather indices are relative to one
of three <=32768-row source ranges.  Gather calls span groups of dst tiles to
amortize SWDGE descriptor-generation overhead.
"""
import sys

sys.path.insert(0, '/opt/trn_rl_repo')

import numpy as np
import jax
from jax.sharding import Mesh, PartitionSpec
from jax.experimental.shard_map import shard_map

import concourse.bass as bass
import concourse.bacc as bacc
import concourse.tile as tile
import concourse.mybir as mybir
from concourse import bass2jax
from concourse.bass2jax import _bass_exec_p, partition_id_tensor
from concourse.masks import make_identity

F32 = mybir.dt.float32
I16 = mybir.dt.int16

N_NODES = 80000
IN_CH = 128
HID = 64
OUT_CH = 32
N_CORES = 8
NT = N_NODES // 128                     # 625 dst tiles
TPC = (NT + N_CORES - 1) // N_CORES     # 79 tile ranks per core
ROWS = TPC * 128                        # 10112 rows per core shard
FULL = N_CORES * ROWS                   # 80896 rows in gathered layout
R_BASES = np.array([0, 32768, 65536], np.int64)
R_SIZES = np.array([32768, 32768, FULL - 65536], np.int64)
R = 3
GROUP = 3                               # dst tiles per gather-call group
CALL_MAX = 896                          # max indices per dma_gather call
                                        # (SWDGE ring holds 1024 descriptors)
S8 = 8                                  # chunks per one-hot build
POOL_NTH = 4                            # every POOL_NTH-th S8 build on gpsimd

# balanced tile->core split: sizes differ by at most 1
_BOUNDS = [round(i * NT / N_CORES) for i in range(N_CORES + 1)]


def _ceil128(x):
    return ((x + 127) // 128) * 128


def _layer_tables(s_ids, d_all, tile_g, core_of_tile, rank_of, ecore):
    """Build per-core gather/scatter tables for one aggregation layer.

    Segment = (dst tile rank t, src range r).  Layout is group-major:
    for each group of GROUP ranks, for each range r, the group's segments
    (t asc) are contiguous, so one dma_gather call covers (group, r).
    Counts are padded to a global (across cores) per-segment max so the
    SPMD program is identical on every core.
    """
    e_r = np.searchsorted(R_BASES[1:], s_ids, side='right')

    # per-core per-(t, r) counts -> global max
    cnt = np.zeros((N_CORES, TPC, R), np.int64)
    for c in range(N_CORES):
        m = ecore == c
        seg = rank_of[c][tile_g[m]] * R + e_r[m]
        cnt[c] = np.bincount(seg, minlength=TPC * R).reshape(TPC, R)
    K = _ceil128(cnt.max(axis=0))           # [TPC, R]

    groups = [(g0, min(g0 + GROUP, TPC)) for g0 in range(0, TPC, GROUP)]
    # segment order: group-major, then range, then tile
    seg_order = []
    for (g0, g1) in groups:
        for r in range(R):
            for t in range(g0, g1):
                seg_order.append((t, r))
    seg_pos = np.zeros(TPC * R, np.int64)
    for i, (t, r) in enumerate(seg_order):
        seg_pos[t * R + r] = i
    K_ord = np.array([K[t, r] for (t, r) in seg_order], np.int64)
    Koff = np.zeros(len(seg_order) + 1, np.int64)
    Koff[1:] = np.cumsum(K_ord)
    T_pad = int(Koff[-1])
    C_total = T_pad // 128

    # chunk -> dst tile rank; per-tile first/last chunk ids
    chunk_tile = np.zeros(C_total, np.int64)
    for i, (t, r) in enumerate(seg_order):
        c0, c1 = Koff[i] // 128, Koff[i + 1] // 128
        chunk_tile[c0:c1] = t
    first_chunk = np.full(TPC, -1, np.int64)
    last_chunk = np.full(TPC, -1, np.int64)
    for j in range(C_total):
        t = chunk_tile[j]
        if first_chunk[t] < 0:
            first_chunk[t] = j
        last_chunk[t] = j

    # gather calls: one span per (group, r), split to CALL_MAX
    calls_of_group = []
    si = 0
    for (g0, g1) in groups:
        calls = []
        for r in range(R):
            n = g1 - g0
            off = int(Koff[si])
            end = int(Koff[si + n])
            si += n
            k = end - off
            while k > 0:
                sz = min(k, CALL_MAX)
                calls.append((r, off, sz))
                off += sz
                k -= sz
        calls_of_group.append(calls)

    # per-core index + dst-label streams
    gidx = np.zeros((N_CORES, T_pad), np.int16)
    dstl = np.full((N_CORES, T_pad), -1.0, np.float32)
    for c in range(N_CORES):
        m = ecore == c
        seg = seg_pos[rank_of[c][tile_g[m]] * R + e_r[m]]
        order = np.argsort(seg, kind='stable')
        seg_s = seg[order]
        seg_counts = np.bincount(seg_s, minlength=len(seg_order))
        starts = np.zeros(len(seg_order), np.int64)
        starts[1:] = np.cumsum(seg_counts)[:-1]
        within = np.arange(len(seg_s)) - starts[seg_s]
        pos = Koff[seg_s] + within
        r_of = np.array([r for (_, r) in seg_order], np.int64)
        gidx[c, pos] = (s_ids[m][order] - R_BASES[r_of[seg_s]]).astype(np.int16)
        dstl[c, pos] = (d_all[m][order] & 127).astype(np.float32)

    # idx table: wrapped in 16 partitions, replicated to 128
    idxw = np.tile(
        gidx.reshape(N_CORES, T_pad // 16, 16).transpose(0, 2, 1), (1, 8, 1)
    ).astype(np.int16)
    # dstv: [cores, 128, C8] chunk dst labels down partitions, pad cols = -1
    C8 = ((C_total + S8 - 1) // S8) * S8
    dstv = np.full((N_CORES, 128, C8), -1.0, np.float32)
    dstv[:, :, :C_total] = dstl.reshape(N_CORES, C_total, 128).transpose(0, 2, 1)

    nchunks = (K.sum(axis=1) // 128).astype(np.int64)
    return dict(K=K, T_pad=T_pad, C_total=C_total, C8=C8,
                groups=groups, calls_of_group=calls_of_group,
                chunk_tile=chunk_tile, first_chunk=first_chunk,
                last_chunk=last_chunk, idxw=idxw, dstv=dstv, nchunks=nchunks)


def _preprocess(edge_index):
    src = np.asarray(edge_index[0], np.int64)
    dst = np.asarray(edge_index[1], np.int64)
    deg = np.bincount(dst, minlength=N_NODES).astype(np.float64) + 1.0
    dinv = (1.0 / np.sqrt(deg)).astype(np.float32)
    loop = np.arange(N_NODES, dtype=np.int64)
    s_all = np.concatenate([src, loop])
    d_all = np.concatenate([dst, loop])
    tile_g = d_all >> 7

    core_of_tile = (np.searchsorted(_BOUNDS, np.arange(NT), side='right') - 1)
    core_of_tile = np.minimum(core_of_tile, N_CORES - 1).astype(np.int64)

    # balanced rank order per core (desc by in-edge count, incl self loops)
    cnt_tile = np.bincount(tile_g, minlength=NT)
    tile_of = -np.ones((N_CORES, TPC), np.int64)
    rank_of = np.full((N_CORES, NT), -1, np.int64)
    for c in range(N_CORES):
        tl = np.where(core_of_tile == c)[0]
        order = tl[np.argsort(-cnt_tile[tl], kind='stable')]
        tile_of[c, :len(order)] = order
        rank_of[c, order] = np.arange(len(order))
    ecore = core_of_tile[tile_g]

    # natural-shard row of node n in zfull (phase A / AG1 layout)
    node_core = np.minimum(
        np.searchsorted(_BOUNDS, np.arange(N_NODES) >> 7, side='right') - 1,
        N_CORES - 1)
    zrow = (node_core * ROWS
            + (np.arange(N_NODES) - np.asarray(_BOUNDS)[node_core] * 128))
    # rank-order row of node n in yfull (L1 output / AG2 layout)
    rank_row_of_tile = np.full(NT, -1, np.int64)
    for c in range(N_CORES):
        for t in range(TPC):
            tl = tile_of[c, t]
            if tl >= 0:
                rank_row_of_tile[tl] = c * TPC + t
    perm_row = (rank_row_of_tile[np.arange(N_NODES) >> 7] * 128
                + (np.arange(N_NODES) & 127))

    L1 = _layer_tables(zrow[s_all], d_all, tile_g, core_of_tile, rank_of, ecore)
    L2 = _layer_tables(perm_row[s_all], d_all, tile_g, core_of_tile, rank_of,
                       ecore)

    # per-core staging vectors
    dinvA = np.ones((N_CORES, 128, TPC), np.float32)   # natural order
    dinvS = np.ones((N_CORES, 128, TPC), np.float32)   # rank order
    for c in range(N_CORES):
        lo, hi = _BOUNDS[c] * 128, _BOUNDS[c + 1] * 128
        nt = (hi - lo) // 128
        dinvA[c, :, :nt] = dinv[lo:hi].reshape(-1, 128).T
        for t in range(TPC):
            tl = tile_of[c, t]
            if tl >= 0:
                dinvS[c, :, t] = dinv[tl * 128:(tl + 1) * 128]

    return dict(dinv=dinv, tile_of=tile_of, L1=L1, L2=L2,
                dinvA=dinvA, dinvS=dinvS)


def _agg_phase(nc, tc, pools, tabs, src_d, oc, idx_sb, dsv_sb, iota8, qn0,
               epilogue):
    """Gather + one-hot scatter-matmul aggregation over one layer."""
    aggp, mp, sp = pools
    groups = tabs["groups"]
    calls_of_group = tabs["calls_of_group"]
    chunk_tile = tabs["chunk_tile"]
    first_chunk = tabs["first_chunk"]
    last_chunk = tabs["last_chunk"]
    nchunks = tabs["nchunks"]

    qn = qn0
    chunk_g = 0
    S8t = None
    pending_epis = []                   # flush one group late: the epilogue
    for gi, (g0, g1) in enumerate(groups):  # chain (DVE/Act/PE/DMA) then
        psums = {}                          # overlaps the next group's gathers
        for t in range(g0, g1):
            if nchunks[t] > 0:
                ps = aggp.tile([128, HID], F32, space="PSUM", tag="agg",
                               name=f"agg_ps_{t}")
                psums[t] = ps
        for (r, off, sz) in calls_of_group[gi]:
            m = mp.tile([128, CALL_MAX // 128, HID], F32, tag="msg",
                        name=f"msg_{gi}_{r}_{off}")
            base = int(R_BASES[r])
            size_r = int(R_SIZES[r])
            nc.gpsimd.dma_gather(
                out_ap=m[:, :sz // 128, :],
                in_ap=src_d.ap()[base:base + size_r, :],
                idxs_ap=idx_sb[:, off // 16:(off + sz) // 16],
                num_idxs=sz,
                num_idxs_reg=sz,
                elem_size=HID,
                single_packet=True,
                queue_num=qn % 4,
            )
            qn += 1
            for s in range(sz // 128):
                if chunk_g % S8 == 0:
                    S8t = sp.tile([128, S8, 128], F32, tag="s8",
                                  name=f"s8_{chunk_g}")
                    dv = dsv_sb[:, chunk_g:chunk_g + S8, None]\
                        .to_broadcast([128, S8, 128])
                    nc.vector.tensor_tensor(out=S8t[:], in0=iota8[:], in1=dv,
                                            op=mybir.AluOpType.is_equal)
                t = int(chunk_tile[chunk_g])
                nc.tensor.matmul(out=psums[t][:, :oc],
                                 lhsT=S8t[:, chunk_g % S8, :],
                                 rhs=m[:, s, :oc],
                                 start=(chunk_g == first_chunk[t]),
                                 stop=(chunk_g == last_chunk[t]))
                chunk_g += 1
        pending_epis.append([(t, psums[t]) for t in range(g0, g1)
                             if t in psums])
        if len(pending_epis) >= 2:
            for t, ps in pending_epis.pop(0):
                epilogue(t, ps)
    for grp in pending_epis:
        for t, ps in grp:
            epilogue(t, ps)
    return qn


_STAGE = "full"                         # dev knob: "a" | "l1" | "full"


def _build_fused(pre, zero_bias=False):
    L1, L2 = pre["L1"], pre["L2"]
    nc = bacc.Bacc("TRN2", target_bir_lowering=False, debug=False,
                   num_devices=N_CORES, num_swdge_queues=4)
    xT_d = nc.dram_tensor("xT", [IN_CH, ROWS], F32, kind="ExternalInput")
    w1_d = nc.dram_tensor("w1", [IN_CH, HID], F32, kind="ExternalInput")
    w2_d = nc.dram_tensor("w2", [HID, OUT_CH], F32, kind="ExternalInput")
    dinvA_d = nc.dram_tensor("dinvA", [128, TPC], F32, kind="ExternalInput")
    dinv2S_d = nc.dram_tensor("dinv2S", [128, TPC], F32, kind="ExternalInput")
    dinvS_d = nc.dram_tensor("dinvS", [128, TPC], F32, kind="ExternalInput")
    bbp1_d = nc.dram_tensor("bbp1", [128, TPC * HID], F32,
                            kind="ExternalInput")
    bb2_d = nc.dram_tensor("bb2", [128, OUT_CH], F32, kind="ExternalInput")
    idx1_d = nc.dram_tensor("idx1", [128, L1["T_pad"] // 16], I16,
                            kind="ExternalInput")
    dsv1_d = nc.dram_tensor("dsv1", [128, L1["C8"]], F32,
                            kind="ExternalInput")
    idx2_d = nc.dram_tensor("idx2", [128, L2["T_pad"] // 16], I16,
                            kind="ExternalInput")
    dsv2_d = nc.dram_tensor("dsv2", [128, L2["C8"]], F32,
                            kind="ExternalInput")
    outp_d = nc.dram_tensor("outp", [ROWS, OUT_CH], F32,
                            kind="ExternalOutput")

    zloc = nc.dram_tensor("zloc", [ROWS, HID], F32, kind="Internal")
    zfull = nc.dram_tensor("zfull", [FULL, HID], F32, kind="Internal",
                           addr_space="Shared")
    yloc = nc.dram_tensor("yloc", [ROWS, HID], F32, kind="Internal")
    yfull = nc.dram_tensor("yfull", [FULL, HID], F32, kind="Internal",
                           addr_space="Shared")

    rg = [list(range(N_CORES))]

    with tile.TileContext(nc) as tc:
        with (
            tc.tile_pool(name="const", bufs=1) as cp,
            tc.tile_pool(name="xin", bufs=3) as xp,
            tc.tile_pool(name="zs", bufs=3) as zp,
            tc.tile_pool(name="msgs", bufs=8) as mp,
            tc.tile_pool(name="s8p", bufs=6) as sp,
            tc.tile_pool(name="ep", bufs=3) as ep,
            tc.tile_pool(name="agg", bufs=6, space="PSUM") as aggp,
            tc.tile_pool(name="ps2", bufs=1, space="PSUM") as ps2,
        ):
            ident = cp.tile([128, 128], F32)
            make_identity(nc, ident[:])
            w1sb = cp.tile([IN_CH, HID], F32)
            nc.sync.dma_start(out=w1sb[:], in_=w1_d.ap()[:, :])
            w2sb = cp.tile([HID, OUT_CH], F32)
            nc.sync.dma_start(out=w2sb[:], in_=w2_d.ap()[:, :])
            dinvA_sb = cp.tile([128, TPC], F32)
            nc.sync.dma_start(out=dinvA_sb[:], in_=dinvA_d.ap()[:, :])
            dinv2_sb = cp.tile([128, TPC], F32)
            nc.sync.dma_start(out=dinv2_sb[:], in_=dinv2S_d.ap()[:, :])
            dinvS_sb = cp.tile([128, TPC], F32)
            nc.sync.dma_start(out=dinvS_sb[:], in_=dinvS_d.ap()[:, :])
            bbp1_sb = cp.tile([128, TPC * HID], F32)
            nc.sync.dma_start(out=bbp1_sb[:], in_=bbp1_d.ap()[:, :])
            bb2_sb = cp.tile([128, OUT_CH], F32)
            nc.sync.dma_start(out=bb2_sb[:], in_=bb2_d.ap()[:, :])
            idx1_sb = cp.tile([128, L1["T_pad"] // 16], I16)
            nc.sync.dma_start(out=idx1_sb[:], in_=idx1_d.ap()[:, :])
            dsv1_sb = cp.tile([128, L1["C8"]], F32)
            nc.sync.dma_start(out=dsv1_sb[:], in_=dsv1_d.ap()[:, :])
            idx2_sb = cp.tile([128, L2["T_pad"] // 16], I16)
            nc.sync.dma_start(out=idx2_sb[:], in_=idx2_d.ap()[:, :])
            dsv2_sb = cp.tile([128, L2["C8"]], F32)
            nc.sync.dma_start(out=dsv2_sb[:], in_=dsv2_d.ap()[:, :])
            iota_i = cp.tile([128, S8 * 128], I16)
            nc.gpsimd.iota(iota_i[:], pattern=[[0, S8], [1, 128]], base=0,
                           channel_multiplier=0)
            iota8 = cp.tile([128, S8, 128], F32)
            nc.vector.tensor_copy(out=iota8[:],
                                  in_=iota_i[:].rearrange("p (c f) -> p c f",
                                                          c=S8))

            # ---- phase A: z1 = dinvA * (x @ W1) -> zloc
            for t in range(TPC):
                xt = xp.tile([IN_CH, 128], F32, tag="x", name=f"xt_{t}")
                nc.sync.dma_start(out=xt[:],
                                  in_=xT_d.ap()[:, t * 128:(t + 1) * 128])
                zps = aggp.tile([128, HID], F32, space="PSUM", tag="agg",
                                name=f"zps_{t}")
                nc.tensor.matmul(out=zps[:], lhsT=xt[:], rhs=w1sb[:],
                                 start=True, stop=True)
                zb = zp.tile([128, HID], F32, tag="zb", name=f"zb_{t}")
                nc.vector.tensor_scalar(out=zb[:], in0=zps[:],
                                        scalar1=dinvA_sb[:, t:t + 1],
                                        scalar2=None,
                                        op0=mybir.AluOpType.mult)
                nc.sync.dma_start(out=zloc.ap()[t * 128:(t + 1) * 128, :],
                                  in_=zb[:])

            # ---- AG1
            nc.gpsimd.collective_compute(
                "AllGather", mybir.AluOpType.bypass, replica_groups=rg,
                ins=[zloc.ap()[:, :]], outs=[zfull.ap()[:, :]])

            # ---- L1 aggregation + epilogue -> yloc
            # yloc columns OUT_CH:HID are never written: the L2 gather reads
            # full 256B rows but the matmul consumes only cols 0:OUT_CH.
            def epi1(t, psum):
                z2 = ep.tile([128, HID], F32, tag="z2", name=f"z2_{t}")
                if zero_bias:
                    nc.scalar.activation(
                        out=z2[:], in_=psum[:],
                        func=mybir.ActivationFunctionType.Relu,
                        scale=dinv2_sb[:, t:t + 1])
                else:
                    t2 = ep.tile([128, HID], F32, tag="t2", name=f"t2_{t}")
                    nc.vector.tensor_tensor(
                        out=t2[:], in0=psum[:],
                        in1=bbp1_sb[:, t * HID:(t + 1) * HID],
                        op=mybir.AluOpType.add)
                    nc.scalar.activation(
                        out=z2[:], in_=t2[:],
                        func=mybir.ActivationFunctionType.Relu,
                        scale=dinv2_sb[:, t:t + 1])
                zT_ps = ps2.tile([HID, 128], F32, space="PSUM", tag="zT",
                                 name=f"zT_ps_{t}")
                nc.tensor.transpose(out=zT_ps[:], in_=z2[:], identity=ident[:])
                zT = ep.tile([HID, 128], F32, tag="zTs", name=f"zT_{t}")
                nc.scalar.activation(out=zT[:], in_=zT_ps[:],
                                     func=mybir.ActivationFunctionType.Copy)
                yps = ps2.tile([128, OUT_CH], F32, space="PSUM", tag="yps",
                               name=f"yps_{t}")
                nc.tensor.matmul(out=yps[:], lhsT=zT[:], rhs=w2sb[:],
                                 start=True, stop=True)
                yt = ep.tile([128, OUT_CH], F32, tag="yt", name=f"yt_{t}")
                nc.scalar.activation(out=yt[:], in_=yps[:],
                                     func=mybir.ActivationFunctionType.Copy)
                nc.sync.dma_start(
                    out=yloc.ap()[t * 128:(t + 1) * 128, 0:OUT_CH],
                    in_=yt[:])

            if _STAGE == "a":
                # consume zfull so the NEFF waits for AG1 completion
                zchk = ep.tile([128, OUT_CH], F32, tag="o1", name="zchk")
                nc.sync.dma_start(out=zchk[:],
                                  in_=zfull.ap()[0:128, 0:OUT_CH])
                nc.sync.dma_start(out=outp_d.ap()[0:128, :], in_=zchk[:])
            else:
                qn = _agg_phase(nc, tc, (aggp, mp, sp), L1, zfull, HID,
                                idx1_sb, dsv1_sb, iota8, 0, epi1)

            if _STAGE == "full":
                # ---- AG2
                nc.gpsimd.collective_compute(
                    "AllGather", mybir.AluOpType.bypass, replica_groups=rg,
                    ins=[yloc.ap()[:, :]], outs=[yfull.ap()[:, :]])

                # ---- L2 aggregation + epilogue -> outp
                def epi2(t, psum):
                    o1 = ep.tile([128, OUT_CH], F32, tag="o1", name=f"o1_{t}")
                    nc.scalar.activation(
                        out=o1[:], in_=psum[:, 0:OUT_CH],
                        func=mybir.ActivationFunctionType.Copy,
                        scale=dinvS_sb[:, t:t + 1])
                    if zero_bias:
                        nc.sync.dma_start(
                            out=outp_d.ap()[t * 128:(t + 1) * 128, :],
                            in_=o1[:])
                        return
                    o = ep.tile([128, OUT_CH], F32, tag="o", name=f"o_{t}")
                    nc.vector.tensor_tensor(out=o[:], in0=o1[:],
                                            in1=bb2_sb[:],
                                            op=mybir.AluOpType.add)
                    nc.sync.dma_start(
                        out=outp_d.ap()[t * 128:(t + 1) * 128, :], in_=o[:])

                _agg_phase(nc, tc, (aggp, mp, sp), L2, yfull, OUT_CH,
                           idx2_sb, dsv2_sb, iota8, qn, epi2)

    nc.compile()
    return nc


class _SpmdRunner:
    def __init__(self, nc, n_cores=N_CORES):
        bass2jax.install_neuronx_cc_hook()
        self.nc = nc
        self.n_cores = n_cores
        in_names, out_names, out_avals = [], [], []
        partition_name = nc.partition_id_tensor.name if nc.partition_id_tensor \
            else None
        for alloc in nc.m.functions[0].allocations:
            if not isinstance(alloc, mybir.MemoryLocationSet):
                continue
            name = alloc.memorylocations[0].name
            if alloc.kind == "ExternalInput":
                if name != partition_name:
                    in_names.append(name)
            elif alloc.kind == "ExternalOutput":
                out_names.append(name)
                out_avals.append(jax.core.ShapedArray(
                    tuple(alloc.tensor_shape), mybir.dt.np(alloc.dtype)))
        self.in_names, self.out_names, self.out_avals = \
            in_names, out_names, out_avals
        n_params = len(in_names)
        n_outs = len(out_avals)
        all_names = list(in_names) + list(out_names)
        if partition_name is not None:
            all_names.append(partition_name)

        def _body(*args):
            operands = list(args)
            if partition_name is not None:
                operands.append(partition_id_tensor())
            outs = _bass_exec_p.bind(
                *operands,
                out_avals=tuple(out_avals),
                in_names=tuple(all_names),
                out_names=tuple(out_names),
                lowering_input_output_aliases=(),
                sim_require_finite=True,
                sim_require_nnan=True,
                nc=nc,
            )
            return tuple(outs)

        devices = jax.devices()[:n_cores]
        assert len(devices) == n_cores, \
            f"need {n_cores} cores, have {len(jax.devices())}"
        self.mesh = Mesh(np.asarray(devices), ("core",))
        in_specs = (PartitionSpec("core"),) * (n_params + n_outs)
        out_specs = (PartitionSpec("core"),) * n_outs
        self.fn = jax.jit(
            shard_map(_body, mesh=self.mesh, in_specs=in_specs,
                      out_specs=out_specs, check_rep=False),
            keep_unused=True,
        )

    def run(self, in_maps):
        concat_in = [
            np.concatenate([np.asarray(in_maps[c][nm])
                            for c in range(self.n_cores)], axis=0)
            for nm in self.in_names
        ]
        concat_zeros = [
            np.zeros((self.n_cores * av.shape[0], *av.shape[1:]), av.dtype)
            for av in self.out_avals
        ]
        outs = self.fn(*(concat_in + concat_zeros))
        jax.block_until_ready(outs)
        res = []
        for c in range(self.n_cores):
            d = {}
            for i, nm in enumerate(self.out_names):
                a = np.asarray(outs[i]).reshape(self.n_cores,
                                                *self.out_avals[i].shape)
                d[nm] = a[c]
            res.append(d)
        return res


_CACHE = {}


def _get_programs(edge_index, zero_bias=False):
    key = (hash(np.asarray(edge_index).tobytes()), bool(zero_bias))
    if key not in _CACHE:
        pre = _preprocess(edge_index)
        ncF = _build_fused(pre, zero_bias=zero_bias)
        _CACHE[key] = (pre, _SpmdRunner(ncF))
    return _CACHE[key]


def _make_in_maps(pre, x, W1, b1, W2, b2):
    dinv = pre["dinv"]
    maps = []
    for c in range(N_CORES):
        lo, hi = _BOUNDS[c] * 128, _BOUNDS[c + 1] * 128
        xT = np.zeros((IN_CH, ROWS), np.float32)
        xT[:, :hi - lo] = x[lo:hi].T
        dinvS_c = pre["dinvS"][c]
        bbp1 = (b1[None, None, :] / dinvS_c.T[:, :, None])  # [TPC, 128, HID]
        bbp1 = bbp1.transpose(1, 0, 2).reshape(128, TPC * HID)
        maps.append({
            "xT": xT, "w1": W1, "w2": W2,
            "dinvA": pre["dinvA"][c],
            "dinv2S": (dinvS_c * dinvS_c).astype(np.float32),
            "dinvS": dinvS_c,
            "bbp1": bbp1.astype(np.float32),
            "bb2": np.tile(b2, (128, 1)).astype(np.float32),
            "idx1": pre["L1"]["idxw"][c], "dsv1": pre["L1"]["dstv"][c],
            "idx2": pre["L2"]["idxw"][c], "dsv2": pre["L2"]["dstv"][c],
        })
    return maps


def kernel(x, edge_index, W1, b1, W2, b2):
    x = np.asarray(x, np.float32)
    W1 = np.asarray(W1, np.float32)
    b1 = np.asarray(b1, np.float32)
    W2 = np.asarray(W2, np.float32)
    b2 = np.asarray(b2, np.float32)
    zero_bias = not (np.any(b1) or np.any(b2))
    pre, runner = _get_programs(edge_index, zero_bias=zero_bias)
    maps = _make_in_maps(pre, x, W1, b1, W2, b2)
    res = runner.run(maps)
    out = np.zeros((N_NODES, OUT_CH), np.float32)
    tile_of = pre["tile_of"]
    for c in range(N_CORES):
        o = res[c]["outp"]
        for t in range(TPC):
            tl = tile_of[c, t]
            if tl >= 0:
                out[tl * 128:(tl + 1) * 128] = o[t * 128:(t + 1) * 128]
    return out


# revision 24
# speedup vs baseline: 1.0604x; 1.0604x over previous
"""GCN 2-layer encoder on 8 Trainium2 NeuronCores (Bass/Tile) — fused single
launch.

kernel(**inputs) takes the FULL inputs and returns the FULL [80000, 32] f32
output.  Strategy (node partition across 8 cores, per sharding hint):

  gcn_conv(x, W, b) = b + dinv * (A_hat @ (dinv * (x @ W)))  with self-loops,
  where dinv = 1/sqrt(indeg+1) and A_hat is the (unnormalized) adjacency.

One SPMD program per core does all of:
  A:   z1 = dinv * (x @ W1) on the core's natural node shard -> zloc (DRAM)
  AG1: AllGather zloc -> zfull  (on-device HBM collective)
  L1:  per dst-node-tile (128 nodes, load-balanced rank order) gather zfull
       rows by edge source (gpsimd dma_gather, 256B rows) and reduce with a
       one-hot scatter-matmul on the PE into PSUM; epilogue
       z2 = relu(dinv^2*agg + dinv*b1);  y = z2 @ W2  -> yloc (DRAM)
  AG2: AllGather yloc -> yfull
  L2:  same aggregation over yfull; epilogue out = dinv*agg + b2.

Cross-partition edge exchange happens entirely on-device via the two
AllGathers, so the whole 2-layer GCN is one kernel launch.  Edges are grouped
by (dst tile, src range) with counts padded to a global per-rank max so all 8
cores run one identical SPMD program; int16 gather indices are relative to one
of three <=32768-row source ranges.  Gather calls span groups of dst tiles to
amortize SWDGE descriptor-generation overhead.
"""
import sys

sys.path.insert(0, '/opt/trn_rl_repo')

import numpy as np
import jax
from jax.sharding import Mesh, PartitionSpec
from jax.experimental.shard_map import shard_map

import concourse.bass as bass
import concourse.bacc as bacc
import concourse.tile as tile
import concourse.mybir as mybir
from concourse import bass2jax
from concourse.bass2jax import _bass_exec_p, partition_id_tensor
from concourse.masks import make_identity

F32 = mybir.dt.float32
I16 = mybir.dt.int16

N_NODES = 80000
IN_CH = 128
HID = 64
OUT_CH = 32
N_CORES = 8
NT = N_NODES // 128                     # 625 dst tiles
TPC = (NT + N_CORES - 1) // N_CORES     # 79 tile ranks per core
ROWS = TPC * 128                        # 10112 rows per core shard
FULL = N_CORES * ROWS                   # 80896 rows in gathered layout
R_BASES = np.array([0, 32768, 65536], np.int64)
R_SIZES = np.array([32768, 32768, FULL - 65536], np.int64)
R = 3
GROUP = 4                               # dst tiles per gather-call group
CALL_MAX = 896                          # max indices per dma_gather call
                                        # (SWDGE ring holds 1024 descriptors)
S8 = 8                                  # chunks per one-hot build
POOL_NTH = 4                            # every POOL_NTH-th S8 build on gpsimd

# balanced tile->core split: sizes differ by at most 1
_BOUNDS = [round(i * NT / N_CORES) for i in range(N_CORES + 1)]


def _ceil128(x):
    return ((x + 127) // 128) * 128


def _layer_tables(s_ids, d_all, tile_g, core_of_tile, rank_of, ecore):
    """Build per-core gather/scatter tables for one aggregation layer.

    Segment = (dst tile rank t, src range r).  Layout is group-major:
    for each group of GROUP ranks, for each range r, the group's segments
    (t asc) are contiguous, so one dma_gather call covers (group, r).
    Counts are padded to a global (across cores) per-segment max so the
    SPMD program is identical on every core.
    """
    e_r = np.searchsorted(R_BASES[1:], s_ids, side='right')

    # per-core per-(t, r) counts -> global max
    cnt = np.zeros((N_CORES, TPC, R), np.int64)
    for c in range(N_CORES):
        m = ecore == c
        seg = rank_of[c][tile_g[m]] * R + e_r[m]
        cnt[c] = np.bincount(seg, minlength=TPC * R).reshape(TPC, R)
    K = _ceil128(cnt.max(axis=0))           # [TPC, R]

    groups = [(g0, min(g0 + GROUP, TPC)) for g0 in range(0, TPC, GROUP)]
    # segment order: group-major, then range, then tile
    seg_order = []
    for (g0, g1) in groups:
        for r in range(R):
            for t in range(g0, g1):
                seg_order.append((t, r))
    seg_pos = np.zeros(TPC * R, np.int64)
    for i, (t, r) in enumerate(seg_order):
        seg_pos[t * R + r] = i
    K_ord = np.array([K[t, r] for (t, r) in seg_order], np.int64)
    Koff = np.zeros(len(seg_order) + 1, np.int64)
    Koff[1:] = np.cumsum(K_ord)
    T_pad = int(Koff[-1])
    C_total = T_pad // 128

    # chunk -> dst tile rank; per-tile first/last chunk ids
    chunk_tile = np.zeros(C_total, np.int64)
    for i, (t, r) in enumerate(seg_order):
        c0, c1 = Koff[i] // 128, Koff[i + 1] // 128
        chunk_tile[c0:c1] = t
    first_chunk = np.full(TPC, -1, np.int64)
    last_chunk = np.full(TPC, -1, np.int64)
    for j in range(C_total):
        t = chunk_tile[j]
        if first_chunk[t] < 0:
            first_chunk[t] = j
        last_chunk[t] = j

    # gather calls: one span per (group, r), split to CALL_MAX
    calls_of_group = []
    si = 0
    for (g0, g1) in groups:
        calls = []
        for r in range(R):
            n = g1 - g0
            off = int(Koff[si])
            end = int(Koff[si + n])
            si += n
            k = end - off
            while k > 0:
                sz = min(k, CALL_MAX)
                calls.append((r, off, sz))
                off += sz
                k -= sz
        calls_of_group.append(calls)

    # per-core index + dst-label streams
    gidx = np.zeros((N_CORES, T_pad), np.int16)
    dstl = np.full((N_CORES, T_pad), -1.0, np.float32)
    for c in range(N_CORES):
        m = ecore == c
        seg = seg_pos[rank_of[c][tile_g[m]] * R + e_r[m]]
        order = np.argsort(seg, kind='stable')
        seg_s = seg[order]
        seg_counts = np.bincount(seg_s, minlength=len(seg_order))
        starts = np.zeros(len(seg_order), np.int64)
        starts[1:] = np.cumsum(seg_counts)[:-1]
        within = np.arange(len(seg_s)) - starts[seg_s]
        pos = Koff[seg_s] + within
        r_of = np.array([r for (_, r) in seg_order], np.int64)
        gidx[c, pos] = (s_ids[m][order] - R_BASES[r_of[seg_s]]).astype(np.int16)
        dstl[c, pos] = (d_all[m][order] & 127).astype(np.float32)

    # idx table: wrapped in 16 partitions, replicated to 128
    idxw = np.tile(
        gidx.reshape(N_CORES, T_pad // 16, 16).transpose(0, 2, 1), (1, 8, 1)
    ).astype(np.int16)
    # dstv: [cores, 128, C8] chunk dst labels down partitions, pad cols = -1
    C8 = ((C_total + S8 - 1) // S8) * S8
    dstv = np.full((N_CORES, 128, C8), -1.0, np.float32)
    dstv[:, :, :C_total] = dstl.reshape(N_CORES, C_total, 128).transpose(0, 2, 1)

    nchunks = (K.sum(axis=1) // 128).astype(np.int64)
    return dict(K=K, T_pad=T_pad, C_total=C_total, C8=C8,
                groups=groups, calls_of_group=calls_of_group,
                chunk_tile=chunk_tile, first_chunk=first_chunk,
                last_chunk=last_chunk, idxw=idxw, dstv=dstv, nchunks=nchunks)


def _preprocess(edge_index):
    src = np.asarray(edge_index[0], np.int64)
    dst = np.asarray(edge_index[1], np.int64)
    deg = np.bincount(dst, minlength=N_NODES).astype(np.float64) + 1.0
    dinv = (1.0 / np.sqrt(deg)).astype(np.float32)
    loop = np.arange(N_NODES, dtype=np.int64)
    s_all = np.concatenate([src, loop])
    d_all = np.concatenate([dst, loop])
    tile_g = d_all >> 7

    core_of_tile = (np.searchsorted(_BOUNDS, np.arange(NT), side='right') - 1)
    core_of_tile = np.minimum(core_of_tile, N_CORES - 1).astype(np.int64)

    # balanced rank order per core (desc by in-edge count, incl self loops)
    cnt_tile = np.bincount(tile_g, minlength=NT)
    tile_of = -np.ones((N_CORES, TPC), np.int64)
    rank_of = np.full((N_CORES, NT), -1, np.int64)
    for c in range(N_CORES):
        tl = np.where(core_of_tile == c)[0]
        order = tl[np.argsort(-cnt_tile[tl], kind='stable')]
        tile_of[c, :len(order)] = order
        rank_of[c, order] = np.arange(len(order))
    ecore = core_of_tile[tile_g]

    # natural-shard row of node n in zfull (phase A / AG1 layout)
    node_core = np.minimum(
        np.searchsorted(_BOUNDS, np.arange(N_NODES) >> 7, side='right') - 1,
        N_CORES - 1)
    zrow = (node_core * ROWS
            + (np.arange(N_NODES) - np.asarray(_BOUNDS)[node_core] * 128))
    # rank-order row of node n in yfull (L1 output / AG2 layout)
    rank_row_of_tile = np.full(NT, -1, np.int64)
    for c in range(N_CORES):
        for t in range(TPC):
            tl = tile_of[c, t]
            if tl >= 0:
                rank_row_of_tile[tl] = c * TPC + t
    perm_row = (rank_row_of_tile[np.arange(N_NODES) >> 7] * 128
                + (np.arange(N_NODES) & 127))

    L1 = _layer_tables(zrow[s_all], d_all, tile_g, core_of_tile, rank_of, ecore)
    L2 = _layer_tables(perm_row[s_all], d_all, tile_g, core_of_tile, rank_of,
                       ecore)

    # per-core staging vectors
    dinvA = np.ones((N_CORES, 128, TPC), np.float32)   # natural order
    dinvS = np.ones((N_CORES, 128, TPC), np.float32)   # rank order
    for c in range(N_CORES):
        lo, hi = _BOUNDS[c] * 128, _BOUNDS[c + 1] * 128
        nt = (hi - lo) // 128
        dinvA[c, :, :nt] = dinv[lo:hi].reshape(-1, 128).T
        for t in range(TPC):
            tl = tile_of[c, t]
            if tl >= 0:
                dinvS[c, :, t] = dinv[tl * 128:(tl + 1) * 128]

    return dict(dinv=dinv, tile_of=tile_of, L1=L1, L2=L2,
                dinvA=dinvA, dinvS=dinvS)


def _agg_phase(nc, tc, pools, tabs, src_d, oc, idx_sb, dsv_sb, iota8, qn0,
               epilogue):
    """Gather + one-hot scatter-matmul aggregation over one layer."""
    aggp, mp, sp = pools
    groups = tabs["groups"]
    calls_of_group = tabs["calls_of_group"]
    chunk_tile = tabs["chunk_tile"]
    first_chunk = tabs["first_chunk"]
    last_chunk = tabs["last_chunk"]
    nchunks = tabs["nchunks"]

    qn = qn0
    chunk_g = 0
    S8t = None
    for gi, (g0, g1) in enumerate(groups):
        psums = {}
        for t in range(g0, g1):
            if nchunks[t] > 0:
                ps = aggp.tile([128, HID], F32, space="PSUM", tag="agg",
                               name=f"agg_ps_{t}")
                psums[t] = ps
        for (r, off, sz) in calls_of_group[gi]:
            m = mp.tile([128, CALL_MAX // 128, HID], F32, tag="msg",
                        name=f"msg_{gi}_{r}_{off}")
            base = int(R_BASES[r])
            size_r = int(R_SIZES[r])
            nc.gpsimd.dma_gather(
                out_ap=m[:, :sz // 128, :],
                in_ap=src_d.ap()[base:base + size_r, :],
                idxs_ap=idx_sb[:, off // 16:(off + sz) // 16],
                num_idxs=sz,
                num_idxs_reg=sz,
                elem_size=HID,
                single_packet=True,
                queue_num=qn % 4,
            )
            qn += 1
            for s in range(sz // 128):
                if chunk_g % S8 == 0:
                    S8t = sp.tile([128, S8, 128], F32, tag="s8",
                                  name=f"s8_{chunk_g}")
                    dv = dsv_sb[:, chunk_g:chunk_g + S8, None]\
                        .to_broadcast([128, S8, 128])
                    nc.vector.tensor_tensor(out=S8t[:], in0=iota8[:], in1=dv,
                                            op=mybir.AluOpType.is_equal)
                t = int(chunk_tile[chunk_g])
                nc.tensor.matmul(out=psums[t][:, :oc],
                                 lhsT=S8t[:, chunk_g % S8, :],
                                 rhs=m[:, s, :oc],
                                 start=(chunk_g == first_chunk[t]),
                                 stop=(chunk_g == last_chunk[t]))
                chunk_g += 1
        for t in range(g0, g1):
            if t in psums:
                epilogue(t, psums[t])
    return qn


_STAGE = "full"                         # dev knob: "a" | "l1" | "full"


def _build_fused(pre, zero_bias=False):
    L1, L2 = pre["L1"], pre["L2"]
    nc = bacc.Bacc("TRN2", target_bir_lowering=False, debug=False,
                   num_devices=N_CORES, num_swdge_queues=4)
    xT_d = nc.dram_tensor("xT", [IN_CH, ROWS], F32, kind="ExternalInput")
    w1_d = nc.dram_tensor("w1", [IN_CH, HID], F32, kind="ExternalInput")
    w2_d = nc.dram_tensor("w2", [HID, OUT_CH], F32, kind="ExternalInput")
    dinvA_d = nc.dram_tensor("dinvA", [128, TPC], F32, kind="ExternalInput")
    dinv2S_d = nc.dram_tensor("dinv2S", [128, TPC], F32, kind="ExternalInput")
    dinvS_d = nc.dram_tensor("dinvS", [128, TPC], F32, kind="ExternalInput")
    bbp1_d = nc.dram_tensor("bbp1", [128, TPC * HID], F32,
                            kind="ExternalInput")
    bb2_d = nc.dram_tensor("bb2", [128, OUT_CH], F32, kind="ExternalInput")
    idx1_d = nc.dram_tensor("idx1", [128, L1["T_pad"] // 16], I16,
                            kind="ExternalInput")
    dsv1_d = nc.dram_tensor("dsv1", [128, L1["C8"]], F32,
                            kind="ExternalInput")
    idx2_d = nc.dram_tensor("idx2", [128, L2["T_pad"] // 16], I16,
                            kind="ExternalInput")
    dsv2_d = nc.dram_tensor("dsv2", [128, L2["C8"]], F32,
                            kind="ExternalInput")
    outp_d = nc.dram_tensor("outp", [ROWS, OUT_CH], F32,
                            kind="ExternalOutput")

    zloc = nc.dram_tensor("zloc", [ROWS, HID], F32, kind="Internal")
    zfull = nc.dram_tensor("zfull", [FULL, HID], F32, kind="Internal",
                           addr_space="Shared")
    yloc = nc.dram_tensor("yloc", [ROWS, HID], F32, kind="Internal")
    yfull = nc.dram_tensor("yfull", [FULL, HID], F32, kind="Internal",
                           addr_space="Shared")

    rg = [list(range(N_CORES))]

    with tile.TileContext(nc) as tc:
        with (
            tc.tile_pool(name="const", bufs=1) as cp,
            tc.tile_pool(name="xin", bufs=3) as xp,
            tc.tile_pool(name="zs", bufs=3) as zp,
            tc.tile_pool(name="msgs", bufs=8) as mp,
            tc.tile_pool(name="s8p", bufs=6) as sp,
            tc.tile_pool(name="ep", bufs=3) as ep,
            tc.tile_pool(name="agg", bufs=6, space="PSUM") as aggp,
            tc.tile_pool(name="ps2", bufs=1, space="PSUM") as ps2,
        ):
            ident = cp.tile([128, 128], F32)
            make_identity(nc, ident[:])
            w1sb = cp.tile([IN_CH, HID], F32)
            nc.sync.dma_start(out=w1sb[:], in_=w1_d.ap()[:, :])
            w2sb = cp.tile([HID, OUT_CH], F32)
            nc.sync.dma_start(out=w2sb[:], in_=w2_d.ap()[:, :])
            dinvA_sb = cp.tile([128, TPC], F32)
            nc.sync.dma_start(out=dinvA_sb[:], in_=dinvA_d.ap()[:, :])
            dinv2_sb = cp.tile([128, TPC], F32)
            nc.sync.dma_start(out=dinv2_sb[:], in_=dinv2S_d.ap()[:, :])
            dinvS_sb = cp.tile([128, TPC], F32)
            nc.sync.dma_start(out=dinvS_sb[:], in_=dinvS_d.ap()[:, :])
            bbp1_sb = cp.tile([128, TPC * HID], F32)
            nc.sync.dma_start(out=bbp1_sb[:], in_=bbp1_d.ap()[:, :])
            bb2_sb = cp.tile([128, OUT_CH], F32)
            nc.sync.dma_start(out=bb2_sb[:], in_=bb2_d.ap()[:, :])
            idx1_sb = cp.tile([128, L1["T_pad"] // 16], I16)
            nc.sync.dma_start(out=idx1_sb[:], in_=idx1_d.ap()[:, :])
            dsv1_sb = cp.tile([128, L1["C8"]], F32)
            nc.sync.dma_start(out=dsv1_sb[:], in_=dsv1_d.ap()[:, :])
            idx2_sb = cp.tile([128, L2["T_pad"] // 16], I16)
            nc.sync.dma_start(out=idx2_sb[:], in_=idx2_d.ap()[:, :])
            dsv2_sb = cp.tile([128, L2["C8"]], F32)
            nc.sync.dma_start(out=dsv2_sb[:], in_=dsv2_d.ap()[:, :])
            iota_i = cp.tile([128, S8 * 128], I16)
            nc.gpsimd.iota(iota_i[:], pattern=[[0, S8], [1, 128]], base=0,
                           channel_multiplier=0)
            iota8 = cp.tile([128, S8, 128], F32)
            nc.vector.tensor_copy(out=iota8[:],
                                  in_=iota_i[:].rearrange("p (c f) -> p c f",
                                                          c=S8))

            # ---- phase A: z1 = dinvA * (x @ W1) -> zloc
            for t in range(TPC):
                xt = xp.tile([IN_CH, 128], F32, tag="x", name=f"xt_{t}")
                nc.sync.dma_start(out=xt[:],
                                  in_=xT_d.ap()[:, t * 128:(t + 1) * 128])
                zps = aggp.tile([128, HID], F32, space="PSUM", tag="agg",
                                name=f"zps_{t}")
                nc.tensor.matmul(out=zps[:], lhsT=xt[:], rhs=w1sb[:],
                                 start=True, stop=True)
                zb = zp.tile([128, HID], F32, tag="zb", name=f"zb_{t}")
                nc.vector.tensor_scalar(out=zb[:], in0=zps[:],
                                        scalar1=dinvA_sb[:, t:t + 1],
                                        scalar2=None,
                                        op0=mybir.AluOpType.mult)
                nc.sync.dma_start(out=zloc.ap()[t * 128:(t + 1) * 128, :],
                                  in_=zb[:])

            # ---- AG1
            nc.gpsimd.collective_compute(
                "AllGather", mybir.AluOpType.bypass, replica_groups=rg,
                ins=[zloc.ap()[:, :]], outs=[zfull.ap()[:, :]])

            # ---- L1 aggregation + epilogue -> yloc
            # yloc columns OUT_CH:HID are never written: the L2 gather reads
            # full 256B rows but the matmul consumes only cols 0:OUT_CH.
            def epi1(t, psum):
                z2 = ep.tile([128, HID], F32, tag="z2", name=f"z2_{t}")
                if zero_bias:
                    nc.scalar.activation(
                        out=z2[:], in_=psum[:],
                        func=mybir.ActivationFunctionType.Relu,
                        scale=dinv2_sb[:, t:t + 1])
                else:
                    t2 = ep.tile([128, HID], F32, tag="t2", name=f"t2_{t}")
                    nc.vector.tensor_tensor(
                        out=t2[:], in0=psum[:],
                        in1=bbp1_sb[:, t * HID:(t + 1) * HID],
                        op=mybir.AluOpType.add)
                    nc.scalar.activation(
                        out=z2[:], in_=t2[:],
                        func=mybir.ActivationFunctionType.Relu,
                        scale=dinv2_sb[:, t:t + 1])
                zT_ps = ps2.tile([HID, 128], F32, space="PSUM", tag="zT",
                                 name=f"zT_ps_{t}")
                nc.tensor.transpose(out=zT_ps[:], in_=z2[:], identity=ident[:])
                zT = ep.tile([HID, 128], F32, tag="zTs", name=f"zT_{t}")
                nc.scalar.activation(out=zT[:], in_=zT_ps[:],
                                     func=mybir.ActivationFunctionType.Copy)
                yps = ps2.tile([128, OUT_CH], F32, space="PSUM", tag="yps",
                               name=f"yps_{t}")
                nc.tensor.matmul(out=yps[:], lhsT=zT[:], rhs=w2sb[:],
                                 start=True, stop=True)
                yt = ep.tile([128, OUT_CH], F32, tag="yt", name=f"yt_{t}")
                nc.scalar.activation(out=yt[:], in_=yps[:],
                                     func=mybir.ActivationFunctionType.Copy)
                nc.sync.dma_start(
                    out=yloc.ap()[t * 128:(t + 1) * 128, 0:OUT_CH],
                    in_=yt[:])

            if _STAGE == "a":
                # consume zfull so the NEFF waits for AG1 completion
                zchk = ep.tile([128, OUT_CH], F32, tag="o1", name="zchk")
                nc.sync.dma_start(out=zchk[:],
                                  in_=zfull.ap()[0:128, 0:OUT_CH])
                nc.sync.dma_start(out=outp_d.ap()[0:128, :], in_=zchk[:])
            else:
                qn = _agg_phase(nc, tc, (aggp, mp, sp), L1, zfull, HID,
                                idx1_sb, dsv1_sb, iota8, 0, epi1)

            if _STAGE == "full":
                # ---- AG2
                nc.gpsimd.collective_compute(
                    "AllGather", mybir.AluOpType.bypass, replica_groups=rg,
                    ins=[yloc.ap()[:, :]], outs=[yfull.ap()[:, :]])

                # ---- L2 aggregation + epilogue -> outp
                def epi2(t, psum):
                    o1 = ep.tile([128, OUT_CH], F32, tag="o1", name=f"o1_{t}")
                    nc.scalar.activation(
                        out=o1[:], in_=psum[:, 0:OUT_CH],
                        func=mybir.ActivationFunctionType.Copy,
                        scale=dinvS_sb[:, t:t + 1])
                    if zero_bias:
                        nc.sync.dma_start(
                            out=outp_d.ap()[t * 128:(t + 1) * 128, :],
                            in_=o1[:])
                        return
                    o = ep.tile([128, OUT_CH], F32, tag="o", name=f"o_{t}")
                    nc.vector.tensor_tensor(out=o[:], in0=o1[:],
                                            in1=bb2_sb[:],
                                            op=mybir.AluOpType.add)
                    nc.sync.dma_start(
                        out=outp_d.ap()[t * 128:(t + 1) * 128, :], in_=o[:])

                _agg_phase(nc, tc, (aggp, mp, sp), L2, yfull, OUT_CH,
                           idx2_sb, dsv2_sb, iota8, qn, epi2)

    nc.compile()
    return nc


class _SpmdRunner:
    def __init__(self, nc, n_cores=N_CORES):
        bass2jax.install_neuronx_cc_hook()
        self.nc = nc
        self.n_cores = n_cores
        in_names, out_names, out_avals = [], [], []
        partition_name = nc.partition_id_tensor.name if nc.partition_id_tensor \
            else None
        for alloc in nc.m.functions[0].allocations:
            if not isinstance(alloc, mybir.MemoryLocationSet):
                continue
            name = alloc.memorylocations[0].name
            if alloc.kind == "ExternalInput":
                if name != partition_name:
                    in_names.append(name)
            elif alloc.kind == "ExternalOutput":
                out_names.append(name)
                out_avals.append(jax.core.ShapedArray(
                    tuple(alloc.tensor_shape), mybir.dt.np(alloc.dtype)))
        self.in_names, self.out_names, self.out_avals = \
            in_names, out_names, out_avals
        n_params = len(in_names)
        n_outs = len(out_avals)
        all_names = list(in_names) + list(out_names)
        if partition_name is not None:
            all_names.append(partition_name)

        def _body(*args):
            operands = list(args)
            if partition_name is not None:
                operands.append(partition_id_tensor())
            outs = _bass_exec_p.bind(
                *operands,
                out_avals=tuple(out_avals),
                in_names=tuple(all_names),
                out_names=tuple(out_names),
                lowering_input_output_aliases=(),
                sim_require_finite=True,
                sim_require_nnan=True,
                nc=nc,
            )
            return tuple(outs)

        devices = jax.devices()[:n_cores]
        assert len(devices) == n_cores, \
            f"need {n_cores} cores, have {len(jax.devices())}"
        self.mesh = Mesh(np.asarray(devices), ("core",))
        in_specs = (PartitionSpec("core"),) * (n_params + n_outs)
        out_specs = (PartitionSpec("core"),) * n_outs
        self.fn = jax.jit(
            shard_map(_body, mesh=self.mesh, in_specs=in_specs,
                      out_specs=out_specs, check_rep=False),
            keep_unused=True,
        )

    def run(self, in_maps):
        concat_in = [
            np.concatenate([np.asarray(in_maps[c][nm])
                            for c in range(self.n_cores)], axis=0)
            for nm in self.in_names
        ]
        concat_zeros = [
            np.zeros((self.n_cores * av.shape[0], *av.shape[1:]), av.dtype)
            for av in self.out_avals
        ]
        outs = self.fn(*(concat_in + concat_zeros))
        jax.block_until_ready(outs)
        res = []
        for c in range(self.n_cores):
            d = {}
            for i, nm in enumerate(self.out_names):
                a = np.asarray(outs[i]).reshape(self.n_cores,
                                                *self.out_avals[i].shape)
                d[nm] = a[c]
            res.append(d)
        return res


_CACHE = {}


def _get_programs(edge_index, zero_bias=False):
    key = (hash(np.asarray(edge_index).tobytes()), bool(zero_bias))
    if key not in _CACHE:
        pre = _preprocess(edge_index)
        ncF = _build_fused(pre, zero_bias=zero_bias)
        _CACHE[key] = (pre, _SpmdRunner(ncF))
    return _CACHE[key]


def _make_in_maps(pre, x, W1, b1, W2, b2):
    dinv = pre["dinv"]
    maps = []
    for c in range(N_CORES):
        lo, hi = _BOUNDS[c] * 128, _BOUNDS[c + 1] * 128
        xT = np.zeros((IN_CH, ROWS), np.float32)
        xT[:, :hi - lo] = x[lo:hi].T
        dinvS_c = pre["dinvS"][c]
        bbp1 = (b1[None, None, :] / dinvS_c.T[:, :, None])  # [TPC, 128, HID]
        bbp1 = bbp1.transpose(1, 0, 2).reshape(128, TPC * HID)
        maps.append({
            "xT": xT, "w1": W1, "w2": W2,
            "dinvA": pre["dinvA"][c],
            "dinv2S": (dinvS_c * dinvS_c).astype(np.float32),
            "dinvS": dinvS_c,
            "bbp1": bbp1.astype(np.float32),
            "bb2": np.tile(b2, (128, 1)).astype(np.float32),
            "idx1": pre["L1"]["idxw"][c], "dsv1": pre["L1"]["dstv"][c],
            "idx2": pre["L2"]["idxw"][c], "dsv2": pre["L2"]["dstv"][c],
        })
    return maps


def kernel(x, edge_index, W1, b1, W2, b2):
    x = np.asarray(x, np.float32)
    W1 = np.asarray(W1, np.float32)
    b1 = np.asarray(b1, np.float32)
    W2 = np.asarray(W2, np.float32)
    b2 = np.asarray(b2, np.float32)
    zero_bias = not (np.any(b1) or np.any(b2))
    pre, runner = _get_programs(edge_index, zero_bias=zero_bias)
    maps = _make_in_maps(pre, x, W1, b1, W2, b2)
    res = runner.run(maps)
    out = np.zeros((N_NODES, OUT_CH), np.float32)
    tile_of = pre["tile_of"]
    for c in range(N_CORES):
        o = res[c]["outp"]
        for t in range(TPC):
            tl = tile_of[c, t]
            if tl >= 0:
                out[tl * 128:(tl + 1) * 128] = o[t * 128:(t + 1) * 128]
    return out
